# revision 3
# baseline (speedup 1.0000x reference)
"""Trainium2 Bass kernel for nn_Concat_84653805404637.

Problem: x [40, 256, 256] f32.  supports = x[:10], queries = x[10:30+10].
out[i*10 + j] = concat(supports[j], queries[i], axis=-1)  -> [300, 256, 512].

Pure data movement (memory regime).  Strategy:
  - SPMD over 8 cores: every core runs the identical program producing
    out_c [40, 256, 512] = 4 queries x 10 supports.
  - Host hands each core the shared 10 support rows plus its 4 query rows
    (cores 6/7 overlap queries so 8*4 >= 30; host drops the overlap).
  - On-device: plain HWDGE DMAs write each core's contiguous output slab;
    query rows are broadcast across the 10 pair slots via step-0 APs.
"""

import numpy as np

import concourse.bass as bass
import concourse.mybir as mybir
from concourse.bass_utils import run_bass_kernel_spmd

N_CORES = 8
NSUP = 10  # support rows actually used by the reference (x0 block + x1 block)
NQ = 4  # queries per core (8*4 = 32 slots for 30 queries; overlap discarded)
T = 256
F = 256
NQ_TOTAL = 30

# core c processes queries QSTART[c] : QSTART[c]+4
QSTART = [0, 4, 8, 12, 16, 20, 24, 26]
# host keeps local pair rows KEEP[c] from each core's [40, ...] output.
# cores 0..6 cover pairs 0..280; core 7 covers 260..300, keep its 280..300.
KEEP = [(0, 40)] * 7 + [(20, 40)]

_CACHE: dict = {}


def _build_kernel() -> bass.Bass:
    nc = bass.Bass("TRN2", target_bir_lowering=False)
    sup = nc.dram_tensor("sup", [NSUP, T, F], mybir.dt.float32, kind="ExternalInput")
    qry = nc.dram_tensor("qry", [NQ, T, F], mybir.dt.float32, kind="ExternalInput")
    out = nc.dram_tensor(
        "out", [NQ * NSUP, T, 2 * F], mybir.dt.float32, kind="ExternalOutput"
    )

    with nc.semaphore("dma_sem") as dma_sem, nc.Block() as block:

        @block.sync
        def _(sync):
            n = 0
            for il in range(NQ):
                # support half: out[il*10:(il+1)*10, :, 0:256] = sup
                sync.dma_start(
                    out[il * NSUP : (il + 1) * NSUP, :, 0:F],
                    sup[:, :, :],
                ).then_inc(dma_sem, 16)
                n += 1
                # query half: out[il*10:(il+1)*10, :, 256:512] = qry[il] bcast
                sync.dma_start(
                    out[il * NSUP : (il + 1) * NSUP, :, F : 2 * F],
                    qry[il][None, :, :].broadcast_to([NSUP, T, F]),
                ).then_inc(dma_sem, 16)
                n += 1
            sync.wait_ge(dma_sem, n * 16)

    return nc


def _get_nc() -> bass.Bass:
    if "nc" not in _CACHE:
        _CACHE["nc"] = _build_kernel()
    return _CACHE["nc"]


def kernel(x: np.ndarray) -> np.ndarray:
    x = np.asarray(x, dtype=np.float32)
    sup = np.ascontiguousarray(x[:NSUP])  # [10, 256, 256]
    queries = np.ascontiguousarray(x[10:])  # [30, 256, 256]

    in_maps = []
    for c in range(N_CORES):
        q0 = QSTART[c]
        in_maps.append(
            {
                "sup": sup,
                "qry": np.ascontiguousarray(queries[q0 : q0 + NQ]),
            }
        )

    nc = _get_nc()
    res = run_bass_kernel_spmd(nc, in_maps, core_ids=list(range(N_CORES)))

    parts = []
    for c in range(N_CORES):
        lo, hi = KEEP[c]
        parts.append(res.results[c]["out"][lo:hi])
    full = np.concatenate(parts, axis=0)
    assert full.shape == (NQ_TOTAL * NSUP, T, 2 * F)
    return full
